# revision 42
# baseline (speedup 1.0000x reference)
"""Trainium2 Bass kernel for nn_BSplineActivation (reflected truncated-power form).

Math: y[b,f] = sum_n B_n(x[b,f]) coeff[f,n], cubic B-splines on the uniform
grid linspace(-1,1,14).  In truncated-power form with u = 6.5(clip(x)+1):
  y = sum_{j=0..12} d_j (u-j)_+^3.
Adding the j=13 term (d_13 = coeff_9/6, zero on u<13) makes
  p(u) = sum_{j=0..13} d_j (u-j)^3 == 0  identically, so for u>6.5 the sum
collapses to minus-side powers of the *reflected* coordinate.  With
ax = |x| (clip unnecessary: planes vanish for ax>=1) and b~_m = m/6.5,
m in {0.5,...,6.5}:
  K_m = min((ax - b~_m)^3, 0)        -- 7 shared planes for BOTH branches
  x <  0:  y = sum_m gA_m K_m        (gA_m = -6.5^3 d_{6.5-m})
  x >= 0:  y = sum_m gB_m K_m        (gB_m = -6.5^3 d_{6.5+m})
Two PSUM chains (P = gB-weighted, M = gA-weighted) of per-feature diagonal
matmuls accumulate both branches; the finish selects P or M by sign(x).

Precision: planes m<=2.5 (small |gK| bound b~^3) run in fp16 end-to-end with
fp16 diagonal weights; planes m>=3.5 run fp32 with fp32r (tf32-rounded)
matmuls.  Measured rel-l2 vs the jax reference: 1.49e-2 (< 2e-2 gate).

Engine schedule (per [128,1024] group tile, cost-model balanced):
  ACT  abs, ungated squares for fp32 planes + plane 2, PSUM->fp16 drains
  DVE  fp16 plane ts/tt chain, fp16-N/fp32r-K for planes 3-4, sign mask,
       copy_predicated
  Pool N/K for planes 5-6
  PE   14 diag matmuls per 512-chunk (fp16 + fp32r, 1 cycle/row)
TimelineSim: 81030 ns/core (baseline kernel: 180808 ns, 2.23x).

Layout: features on partitions (8 groups of 128, processed 2 per [128,2048]
tile), batch on the free dim; pure data-parallel over batch across 8 cores.
Host sends per-core shards transposed and cast to fp16 (halves input DMA)
and pre-builds all diagonal weight tiles (zero engine cost, loaded once).
"""

import os
from math import comb

import numpy as np

import concourse.bacc as bacc
import concourse.bass as bass
import concourse.mybir as mybir
import concourse.tile as tile
from concourse.bass_utils import run_bass_kernel_spmd

N_CORES = 8
B_FULL, F = 8192, 1024
B_CORE = B_FULL // N_CORES  # 1024
P = 128
G = F // P  # 8
GPT = 1                      # groups per tile
NT = G // GPT                # 4 tiles per core
W = GPT * B_CORE             # 2048 free-dim cols per tile
CHUNK = 512
PSUM_BUFS = 2
CT_LATE = True
STT_POOL_CUBE = False  # stt on Pool is ISA-invalid on real codegen
FIRST_ABS_DVE = False
NPLANES = 7
BT = [(2 * i + 1) / 13.0 for i in range(NPLANES)]  # (i+0.5)/6.5
FP16_PLANES = (0, 1, 2)
FP32_PLANES = (3, 4, 5, 6)

FP32 = mybir.dt.float32
FP16 = mybir.dt.float16
F32R = mybir.dt.float32r
U8 = mybir.dt.uint8
I16 = mybir.dt.int16

Alu = mybir.AluOpType
Act = mybir.ActivationFunctionType

# fp32-plane engine routing: (N-producer, K-producer); S is always ACT.
# N "act" uses Relu(bt-ax) = -N; the host negates that plane's diag weights.
ROUTE32 = {
    3: ("dve16", "dve"),
    4: ("dve16", "dve"),
    5: ("pool", "pool"),
    6: ("pool", "pool"),
}
S16_ACT = (2,)
FIRST_SLIVER = False
LAST_TILE_DVE = False
CHAIN_ORDER = "interleaved"  # or "pm"
SPLIT_YDMA = True
MASK_EARLY = True
EMIT_ORDER = (0, 1, 2, 3, 4, 5, 6)
BUFS_X = 3
BUFS_W = 3
BUFS_Y = 3
BUFS_PL = 2
BUFS_TR = 2
BUFS_K32 = 2        # fp16 planes whose square runs on ACT (ungated from ax)
MASK = "dve"          # sign mask: "dve"|"pool" (is_ge) or "act" (relu, bitcast)
NEG32 = tuple(i for i in FP32_PLANES if ROUTE32[i][0] == "act")
_CACHE: dict = {}


def _build_nc() -> bass.Bass:
    nc = bacc.Bacc("TRN2", target_bir_lowering=False, debug=False)

    xT = nc.dram_tensor("xT", [F, B_CORE], FP16, kind="ExternalInput")
    # host-packed diagonal weights for ALL groups, partition-major:
    # d16: [P, G*6*P] fp16   (per group: P,M diag pairs for planes 0..2)
    # d32: [P, G*8*P] fp32r  (per group: P,M diag pairs for planes 3..6)
    d16 = nc.dram_tensor("d16", [G, P, 6 * P], FP16, kind="ExternalInput")
    d32 = nc.dram_tensor("d32", [G, P, 8 * P], F32R, kind="ExternalInput")
    cst = nc.dram_tensor("cst", [P, 8], FP32, kind="ExternalInput")
    yT = nc.dram_tensor("yT", [F, B_CORE], FP16, kind="ExternalOutput")

    with tile.TileContext(nc) as tc:
        with (
            tc.tile_pool(name="const", bufs=1) as const_pool,
            tc.tile_pool(name="xdata", bufs=BUFS_X) as x_pool,
            tc.tile_pool(name="wts", bufs=BUFS_W) as w_pool,
            tc.tile_pool(name="trans", bufs=BUFS_TR) as tr_pool,
            tc.tile_pool(name="k32p", bufs=BUFS_K32) as k32_pool,
            tc.tile_pool(name="plane", bufs=BUFS_PL) as pl_pool,
            tc.tile_pool(name="yout", bufs=BUFS_Y) as y_pool,
            tc.tile_pool(name="psum", bufs=PSUM_BUFS, space="PSUM") as psum_pool,
        ):
            ctall = const_pool.tile([P, 8], FP32, name="ctall")
            if not CT_LATE:
                nc.sync.dma_start(ctall[:], cst[:])



            for t in range(NT):
                g0 = t * GPT
                x16 = x_pool.tile([P, W], FP16, name="x16", tag="x16")
                if t == 0 and FIRST_SLIVER:
                    # sliver the first load so compute starts sooner
                    nc.sync.dma_start(x16[:, : W // 4], xT[g0 * P : (g0 + 1) * P, : W // 4])
                    nc.sync.dma_start(x16[:, W // 4 :], xT[g0 * P : (g0 + 1) * P, W // 4 :])
                else:
                    nc.sync.dma_start(
                        x16[:].rearrange("p (gl b) -> p gl b", gl=GPT),
                        xT[g0 * P : (g0 + GPT) * P, :].rearrange(
                            "(gl p) b -> p gl b", p=P
                        ),
                    )
                if CT_LATE and t == 0:
                    nc.sync.dma_start(ctall[:], cst[:])
                dg16 = w_pool.tile([P, 6 * P], FP16, name="dg16", tag="dg16")
                nc.sync.dma_start(dg16[:], d16[g0])
                dg32 = w_pool.tile([P, 8 * P], F32R, name="dg32", tag="dg32")
                nc.sync.dma_start(dg32[:], d32[g0])

                def dP(g, i):
                    if i in FP16_PLANES:
                        return dg16[:, (2 * i) * P : (2 * i + 1) * P]
                    c = 2 * (i - 3) * P
                    return dg32[:, c : c + P]

                def dM(g, i):
                    if i in FP16_PLANES:
                        return dg16[:, (2 * i + 1) * P : (2 * i + 2) * P]
                    c = (2 * (i - 3) + 1) * P
                    return dg32[:, c : c + P]

                ax = pl_pool.tile([P, W], FP16, name="ax", tag="ax")
                if t == 0 and FIRST_ABS_DVE:
                    negx = tr_pool.tile([P, W], FP16, name="negx", tag="negx")
                    nc.vector.tensor_scalar(negx[:], x16[:], -1.0, None, Alu.mult)
                    nc.vector.tensor_max(ax[:], x16[:], negx[:])
                elif t == 0 and FIRST_SLIVER:
                    nc.scalar.activation(ax[:, : W // 4], x16[:, : W // 4], Act.Abs)
                    nc.scalar.activation(ax[:, W // 4 :], x16[:, W // 4 :], Act.Abs)
                else:
                    nc.scalar.activation(ax[:], x16[:], Act.Abs)

                K = {}

                def emit16(i):
                    n = tr_pool.tile([P, W], FP16, name=f"n{i}", tag=f"n{i}")
                    nc.vector.tensor_scalar(
                        n[:], ax[:], BT[i], 0.0, Alu.subtract, Alu.min
                    )
                    s = tr_pool.tile([P, W], FP16, name=f"s{i}", tag=f"s{i}")
                    if i in S16_ACT:
                        nc.scalar.activation(
                            s[:], ax[:], Act.Square,
                            bias=ctall[:, 4 + i : 5 + i], scale=-1.0,
                        )
                    else:
                        nc.vector.tensor_tensor(s[:], n[:], n[:], Alu.mult)
                    k = pl_pool.tile([P, W], FP16, name=f"k{i}", tag=f"k{i}")
                    nc.vector.tensor_tensor(k[:], s[:], n[:], Alu.mult)
                    K[i] = k
                if MASK_EARLY:
                    g8 = pl_pool.tile([P, W], I16, name="g8", tag="g8")
                    geng = nc.vector if MASK == "dve" else nc.gpsimd
                    geng.tensor_scalar(g8[:], x16[:], 0.0, 1.0, Alu.is_ge, Alu.mult)
                def emit32(i):
                    rn, rk = ROUTE32[i]
                    ndt = FP16 if rn == "dve16" else FP32
                    n = tr_pool.tile([P, W], ndt, name=f"n{i}", tag=f"n{i}")
                    if rn == "act":
                        # n = relu(bt - ax) = -N ; diag weights negated on host
                        nc.scalar.activation(
                            n[:], ax[:], Act.Relu,
                            bias=ctall[:, i - 3 : i - 2], scale=-1.0,
                        )
                    elif rn == "splitdp":
                        h = W // 2
                        nc.vector.tensor_scalar(
                            n[:, :h], ax[:, :h], BT[i], 0.0, Alu.subtract, Alu.min
                        )
                        nc.gpsimd.tensor_scalar(
                            n[:, h:], ax[:, h:], BT[i], 0.0, Alu.subtract, Alu.min
                        )
                    else:
                        eng = nc.gpsimd if rn == "pool" else nc.vector
                        eng.tensor_scalar(n[:], ax[:], BT[i], 0.0, Alu.subtract, Alu.min)
                    s = tr_pool.tile([P, W], FP32, name=f"s{i}", tag=f"s{i}")
                    nc.scalar.activation(
                        s[:], ax[:], Act.Square, bias=ctall[:, i - 3 : i - 2], scale=-1.0
                    )
                    k = k32_pool.tile([P, W], F32R, name=f"k{i}", tag=f"k{i}")
                    if rk == "pool" and STT_POOL_CUBE:
                        # stt lowers to TensorScalarPtr: Pool efficiency 0.6
                        # instead of TensorTensor-Multiply's 0.42
                        nc.gpsimd.scalar_tensor_tensor(
                            k[:], s[:], 1.0, n[:], Alu.mult, Alu.mult
                        )
                    else:
                        eng = nc.vector if rk == "dve" else nc.gpsimd
                        eng.tensor_tensor(k[:], s[:], n[:], Alu.mult)
                    K[i] = k

                for i in EMIT_ORDER:
                    (emit16 if i in FP16_PLANES else emit32)(i)

                if not MASK_EARLY:
                    if MASK == "act":
                        g8 = pl_pool.tile([P, W], FP16, name="g8", tag="g8")
                        nc.scalar.activation(g8[:], x16[:], Act.Relu)
                    else:
                        g8 = pl_pool.tile([P, W], I16, name="g8", tag="g8")
                        geng = nc.vector if MASK == "dve" else nc.gpsimd
                        geng.tensor_scalar(g8[:], x16[:], 0.0, 1.0, Alu.is_ge, Alu.mult)
                y16 = y_pool.tile([P, W], FP16, name="y16", tag="y16")

                for ch in range(W // CHUNK):
                    gl = ch // (B_CORE // CHUNK)     # which group in the tile
                    g = g0 + gl
                    Pp = psum_pool.tile([P, CHUNK], FP32, name=f"Pp{ch}", tag=f"Pp{ch}")
                    Mp = psum_pool.tile([P, CHUNK], FP32, name=f"Mp{ch}", tag=f"Mp{ch}")
                    sl = slice(ch * CHUNK, (ch + 1) * CHUNK)
                    if CHAIN_ORDER == "interleaved":
                        for i in range(NPLANES):
                            nc.tensor.matmul(
                                Pp[:], dP(g, i), K[i][:, sl],
                                start=(i == 0), stop=(i == NPLANES - 1),
                            )
                            nc.tensor.matmul(
                                Mp[:], dM(g, i), K[i][:, sl],
                                start=(i == 0), stop=(i == NPLANES - 1),
                            )
                    else:
                        for i in range(NPLANES):
                            nc.tensor.matmul(
                                Pp[:], dP(g, i), K[i][:, sl],
                                start=(i == 0), stop=(i == NPLANES - 1),
                            )
                        for i in range(NPLANES):
                            nc.tensor.matmul(
                                Mp[:], dM(g, i), K[i][:, sl],
                                start=(i == 0), stop=(i == NPLANES - 1),
                            )
                    nc.scalar.copy(y16[:, sl], Mp[:])
                    nc.vector.copy_predicated(
                        y16[:, sl],
                        g8[:, sl].bitcast(I16) if MASK == "act" else g8[:, sl],
                        Pp[:],
                    )

                if SPLIT_YDMA == "quarters":
                    q = W // 4
                    for qi in range(4):
                        nc.sync.dma_start(
                            yT[g0 * P : (g0 + 1) * P, qi * q : (qi + 1) * q],
                            y16[:, qi * q : (qi + 1) * q],
                        )
                elif SPLIT_YDMA:
                    h = W // 2
                    nc.sync.dma_start(yT[g0 * P : (g0 + 1) * P, :h], y16[:, :h])
                    nc.sync.dma_start(yT[g0 * P : (g0 + 1) * P, h:], y16[:, h:])
                else:
                    nc.sync.dma_start(
                        yT[g0 * P : (g0 + GPT) * P, :].rearrange(
                            "(gl p) b -> p gl b", p=P
                        ),
                        y16[:].rearrange("p (gl b) -> p gl b", gl=GPT),
                    )
    nc.compile()
    return nc


def _weights(coeff: np.ndarray):
    d = np.zeros((14, F), dtype=np.float64)
    c64 = coeff.astype(np.float64)
    for j in range(14):
        for r in range(5):
            n = j - r
            if 0 <= n < 10:
                d[j] += (-1) ** r * comb(4, r) / 6.0 * c64[:, n]
    s = 6.5 ** 3
    gA = np.zeros((NPLANES, F))
    gB = np.zeros((NPLANES, F))
    for i in range(NPLANES):
        gA[i] = -s * d[6 - i]
        gB[i] = -s * d[7 + i]
    d16 = np.zeros((G, P, 6 * P), dtype=np.float16)
    d32 = np.zeros((G, P, 8 * P), dtype=np.float32)
    rng = np.arange(P)
    for g in range(G):
        fsl = slice(g * P, (g + 1) * P)
        for i in FP16_PLANES:
            d16[g, rng, 2 * i * P + rng] = gB[i, fsl].astype(np.float16)
            d16[g, rng, (2 * i + 1) * P + rng] = gA[i, fsl].astype(np.float16)
        for i in FP32_PLANES:
            sgn = -1.0 if i in NEG32 else 1.0
            d32[g, rng, 2 * (i - 3) * P + rng] = (sgn * gB[i, fsl]).astype(np.float32)
            d32[g, rng, (2 * (i - 3) + 1) * P + rng] = (sgn * gA[i, fsl]).astype(np.float32)
    cst = np.zeros((P, 8), dtype=np.float32)
    cst[:, 0:4] = np.array(BT[3:], dtype=np.float32)
    cst[:, 4:7] = np.array(BT[:3], dtype=np.float32)
    return d16, d32, cst


def kernel(x: np.ndarray, coeff: np.ndarray) -> np.ndarray:
    x = np.ascontiguousarray(x, dtype=np.float32)
    coeff = np.ascontiguousarray(coeff, dtype=np.float32)
    assert x.shape == (B_FULL, F) and coeff.shape == (F, 10)

    if "nc" not in _CACHE:
        _CACHE["nc"] = _build_nc()
    nc = _CACHE["nc"]

    d16, d32, cst = _weights(coeff)

    in_maps = []
    for c in range(N_CORES):
        shard = np.ascontiguousarray(
            x[c * B_CORE : (c + 1) * B_CORE, :].T.astype(np.float16)
        )
        in_maps.append({"xT": shard, "d16": d16, "d32": d32, "cst": cst})

    trace = os.environ.get("BSPLINE_TRACE", "0") == "1"
    res = run_bass_kernel_spmd(
        nc, in_maps, core_ids=list(range(N_CORES)), trace=trace
    )
    _CACHE["last_result"] = res

    y = np.empty((B_FULL, F), dtype=np.float32)
    for c in range(N_CORES):
        y[c * B_CORE : (c + 1) * B_CORE, :] = res.results[c]["yT"].T.astype(np.float32)
    return y


# revision 43
# speedup vs baseline: 1.0044x; 1.0044x over previous
"""Trainium2 Bass kernel for nn_BSplineActivation (reflected truncated-power form).

Math: y[b,f] = sum_n B_n(x[b,f]) coeff[f,n], cubic B-splines on the uniform
grid linspace(-1,1,14).  In truncated-power form with u = 6.5(clip(x)+1):
  y = sum_{j=0..12} d_j (u-j)_+^3.
Adding the j=13 term (d_13 = coeff_9/6, zero on u<13) makes
  p(u) = sum_{j=0..13} d_j (u-j)^3 == 0  identically, so for u>6.5 the sum
collapses to minus-side powers of the *reflected* coordinate.  With
ax = |x| (clip unnecessary: planes vanish for ax>=1) and b~_m = m/6.5,
m in {0.5,...,6.5}:
  K_m = min((ax - b~_m)^3, 0)        -- 7 shared planes for BOTH branches
  x <  0:  y = sum_m gA_m K_m        (gA_m = -6.5^3 d_{6.5-m})
  x >= 0:  y = sum_m gB_m K_m        (gB_m = -6.5^3 d_{6.5+m})
Two PSUM chains (P = gB-weighted, M = gA-weighted) of per-feature diagonal
matmuls accumulate both branches; the finish selects P or M by sign(x).

Precision: planes m<=2.5 (small |gK| bound b~^3) run in fp16 end-to-end with
fp16 diagonal weights; planes m>=3.5 run fp32 with fp32r (tf32-rounded)
matmuls.  Measured rel-l2 vs the jax reference: 1.49e-2 (< 2e-2 gate).

Engine schedule (per [128,1024] group tile, cost-model balanced):
  ACT  abs, ungated squares for fp32 planes + plane 2, PSUM->fp16 drains
  DVE  fp16 plane ts/tt chain, fp16-N/fp32r-K for planes 3-4, sign mask,
       copy_predicated
  Pool N/K for planes 5-6
  PE   14 diag matmuls per 512-chunk (fp16 + fp32r, 1 cycle/row)
TimelineSim: 80674 ns/core (baseline kernel: 180808 ns, 2.24x).

Layout: features on partitions (8 groups of 128, processed 2 per [128,2048]
tile), batch on the free dim; pure data-parallel over batch across 8 cores.
Host sends per-core shards transposed and cast to fp16 (halves input DMA)
and pre-builds all diagonal weight tiles (zero engine cost, loaded once).
"""

import os
from math import comb

import numpy as np

import concourse.bacc as bacc
import concourse.bass as bass
import concourse.mybir as mybir
import concourse.tile as tile
from concourse.bass_utils import run_bass_kernel_spmd

N_CORES = 8
B_FULL, F = 8192, 1024
B_CORE = B_FULL // N_CORES  # 1024
P = 128
G = F // P  # 8
GPT = 1                      # groups per tile
NT = G // GPT                # 4 tiles per core
W = GPT * B_CORE             # 2048 free-dim cols per tile
CHUNK = 512
PSUM_BUFS = 2
CT_LATE = True
STT_POOL_CUBE = False  # stt on Pool is ISA-invalid on real codegen
FIRST_ABS_DVE = False
NPLANES = 7
BT = [(2 * i + 1) / 13.0 for i in range(NPLANES)]  # (i+0.5)/6.5
FP16_PLANES = (0, 1, 2)
FP32_PLANES = (3, 4, 5, 6)

FP32 = mybir.dt.float32
FP16 = mybir.dt.float16
F32R = mybir.dt.float32r
U8 = mybir.dt.uint8
I16 = mybir.dt.int16

Alu = mybir.AluOpType
Act = mybir.ActivationFunctionType

# fp32-plane engine routing: (N-producer, K-producer); S is always ACT.
# N "act" uses Relu(bt-ax) = -N; the host negates that plane's diag weights.
ROUTE32 = {
    3: ("dve16", "dve"),
    4: ("dve16", "dve"),
    5: ("pool", "pool"),
    6: ("pool", "pool"),
}
S16_ACT = (2,)
FIRST_SLIVER = False
LAST_TILE_DVE = False
CHAIN_ORDER = "interleaved"  # or "pm"
SPLIT_YDMA = True
MASK_EARLY = True
EMIT_ORDER = (0, 1, 2, 3, 4, 5, 6)
BUFS_X = 3
BUFS_W = 3
BUFS_Y = 3
BUFS_PL = 3
BUFS_TR = 2
BUFS_K32 = 2        # fp16 planes whose square runs on ACT (ungated from ax)
MASK = "dve"          # sign mask: "dve"|"pool" (is_ge) or "act" (relu, bitcast)
NEG32 = tuple(i for i in FP32_PLANES if ROUTE32[i][0] == "act")
_CACHE: dict = {}


def _build_nc() -> bass.Bass:
    nc = bacc.Bacc("TRN2", target_bir_lowering=False, debug=False)

    xT = nc.dram_tensor("xT", [F, B_CORE], FP16, kind="ExternalInput")
    # host-packed diagonal weights for ALL groups, partition-major:
    # d16: [P, G*6*P] fp16   (per group: P,M diag pairs for planes 0..2)
    # d32: [P, G*8*P] fp32r  (per group: P,M diag pairs for planes 3..6)
    d16 = nc.dram_tensor("d16", [G, P, 6 * P], FP16, kind="ExternalInput")
    d32 = nc.dram_tensor("d32", [G, P, 8 * P], F32R, kind="ExternalInput")
    cst = nc.dram_tensor("cst", [P, 8], FP32, kind="ExternalInput")
    yT = nc.dram_tensor("yT", [F, B_CORE], FP16, kind="ExternalOutput")

    with tile.TileContext(nc) as tc:
        with (
            tc.tile_pool(name="const", bufs=1) as const_pool,
            tc.tile_pool(name="xdata", bufs=BUFS_X) as x_pool,
            tc.tile_pool(name="wts", bufs=BUFS_W) as w_pool,
            tc.tile_pool(name="trans", bufs=BUFS_TR) as tr_pool,
            tc.tile_pool(name="k32p", bufs=BUFS_K32) as k32_pool,
            tc.tile_pool(name="plane", bufs=BUFS_PL) as pl_pool,
            tc.tile_pool(name="yout", bufs=BUFS_Y) as y_pool,
            tc.tile_pool(name="psum", bufs=PSUM_BUFS, space="PSUM") as psum_pool,
        ):
            ctall = const_pool.tile([P, 8], FP32, name="ctall")
            if not CT_LATE:
                nc.sync.dma_start(ctall[:], cst[:])



            for t in range(NT):
                g0 = t * GPT
                x16 = x_pool.tile([P, W], FP16, name="x16", tag="x16")
                if t == 0 and FIRST_SLIVER:
                    # sliver the first load so compute starts sooner
                    nc.sync.dma_start(x16[:, : W // 4], xT[g0 * P : (g0 + 1) * P, : W // 4])
                    nc.sync.dma_start(x16[:, W // 4 :], xT[g0 * P : (g0 + 1) * P, W // 4 :])
                else:
                    nc.sync.dma_start(
                        x16[:].rearrange("p (gl b) -> p gl b", gl=GPT),
                        xT[g0 * P : (g0 + GPT) * P, :].rearrange(
                            "(gl p) b -> p gl b", p=P
                        ),
                    )
                if CT_LATE and t == 0:
                    nc.sync.dma_start(ctall[:], cst[:])
                dg16 = w_pool.tile([P, 6 * P], FP16, name="dg16", tag="dg16")
                nc.sync.dma_start(dg16[:], d16[g0])
                dg32 = w_pool.tile([P, 8 * P], F32R, name="dg32", tag="dg32")
                nc.sync.dma_start(dg32[:], d32[g0])

                def dP(g, i):
                    if i in FP16_PLANES:
                        return dg16[:, (2 * i) * P : (2 * i + 1) * P]
                    c = 2 * (i - 3) * P
                    return dg32[:, c : c + P]

                def dM(g, i):
                    if i in FP16_PLANES:
                        return dg16[:, (2 * i + 1) * P : (2 * i + 2) * P]
                    c = (2 * (i - 3) + 1) * P
                    return dg32[:, c : c + P]

                ax = pl_pool.tile([P, W], FP16, name="ax", tag="ax")
                if t == 0 and FIRST_ABS_DVE:
                    negx = tr_pool.tile([P, W], FP16, name="negx", tag="negx")
                    nc.vector.tensor_scalar(negx[:], x16[:], -1.0, None, Alu.mult)
                    nc.vector.tensor_max(ax[:], x16[:], negx[:])
                elif t == 0 and FIRST_SLIVER:
                    nc.scalar.activation(ax[:, : W // 4], x16[:, : W // 4], Act.Abs)
                    nc.scalar.activation(ax[:, W // 4 :], x16[:, W // 4 :], Act.Abs)
                else:
                    nc.scalar.activation(ax[:], x16[:], Act.Abs)

                K = {}

                def emit16(i):
                    n = tr_pool.tile([P, W], FP16, name=f"n{i}", tag=f"n{i}")
                    nc.vector.tensor_scalar(
                        n[:], ax[:], BT[i], 0.0, Alu.subtract, Alu.min
                    )
                    s = tr_pool.tile([P, W], FP16, name=f"s{i}", tag=f"s{i}")
                    if i in S16_ACT:
                        nc.scalar.activation(
                            s[:], ax[:], Act.Square,
                            bias=ctall[:, 4 + i : 5 + i], scale=-1.0,
                        )
                    else:
                        nc.vector.tensor_tensor(s[:], n[:], n[:], Alu.mult)
                    k = pl_pool.tile([P, W], FP16, name=f"k{i}", tag=f"k{i}")
                    nc.vector.tensor_tensor(k[:], s[:], n[:], Alu.mult)
                    K[i] = k
                if MASK_EARLY:
                    g8 = pl_pool.tile([P, W], I16, name="g8", tag="g8")
                    geng = nc.vector if MASK == "dve" else nc.gpsimd
                    geng.tensor_scalar(g8[:], x16[:], 0.0, 1.0, Alu.is_ge, Alu.mult)
                def emit32(i):
                    rn, rk = ROUTE32[i]
                    ndt = FP16 if rn == "dve16" else FP32
                    n = tr_pool.tile([P, W], ndt, name=f"n{i}", tag=f"n{i}")
                    if rn == "act":
                        # n = relu(bt - ax) = -N ; diag weights negated on host
                        nc.scalar.activation(
                            n[:], ax[:], Act.Relu,
                            bias=ctall[:, i - 3 : i - 2], scale=-1.0,
                        )
                    elif rn == "splitdp":
                        h = W // 2
                        nc.vector.tensor_scalar(
                            n[:, :h], ax[:, :h], BT[i], 0.0, Alu.subtract, Alu.min
                        )
                        nc.gpsimd.tensor_scalar(
                            n[:, h:], ax[:, h:], BT[i], 0.0, Alu.subtract, Alu.min
                        )
                    else:
                        eng = nc.gpsimd if rn == "pool" else nc.vector
                        eng.tensor_scalar(n[:], ax[:], BT[i], 0.0, Alu.subtract, Alu.min)
                    s = tr_pool.tile([P, W], FP32, name=f"s{i}", tag=f"s{i}")
                    nc.scalar.activation(
                        s[:], ax[:], Act.Square, bias=ctall[:, i - 3 : i - 2], scale=-1.0
                    )
                    k = k32_pool.tile([P, W], F32R, name=f"k{i}", tag=f"k{i}")
                    if rk == "pool" and STT_POOL_CUBE:
                        # stt lowers to TensorScalarPtr: Pool efficiency 0.6
                        # instead of TensorTensor-Multiply's 0.42
                        nc.gpsimd.scalar_tensor_tensor(
                            k[:], s[:], 1.0, n[:], Alu.mult, Alu.mult
                        )
                    else:
                        eng = nc.vector if rk == "dve" else nc.gpsimd
                        eng.tensor_tensor(k[:], s[:], n[:], Alu.mult)
                    K[i] = k

                for i in EMIT_ORDER:
                    (emit16 if i in FP16_PLANES else emit32)(i)

                if not MASK_EARLY:
                    if MASK == "act":
                        g8 = pl_pool.tile([P, W], FP16, name="g8", tag="g8")
                        nc.scalar.activation(g8[:], x16[:], Act.Relu)
                    else:
                        g8 = pl_pool.tile([P, W], I16, name="g8", tag="g8")
                        geng = nc.vector if MASK == "dve" else nc.gpsimd
                        geng.tensor_scalar(g8[:], x16[:], 0.0, 1.0, Alu.is_ge, Alu.mult)
                y16 = y_pool.tile([P, W], FP16, name="y16", tag="y16")

                for ch in range(W // CHUNK):
                    gl = ch // (B_CORE // CHUNK)     # which group in the tile
                    g = g0 + gl
                    Pp = psum_pool.tile([P, CHUNK], FP32, name=f"Pp{ch}", tag=f"Pp{ch}")
                    Mp = psum_pool.tile([P, CHUNK], FP32, name=f"Mp{ch}", tag=f"Mp{ch}")
                    sl = slice(ch * CHUNK, (ch + 1) * CHUNK)
                    if CHAIN_ORDER == "interleaved":
                        for i in range(NPLANES):
                            nc.tensor.matmul(
                                Pp[:], dP(g, i), K[i][:, sl],
                                start=(i == 0), stop=(i == NPLANES - 1),
                            )
                            nc.tensor.matmul(
                                Mp[:], dM(g, i), K[i][:, sl],
                                start=(i == 0), stop=(i == NPLANES - 1),
                            )
                    else:
                        for i in range(NPLANES):
                            nc.tensor.matmul(
                                Pp[:], dP(g, i), K[i][:, sl],
                                start=(i == 0), stop=(i == NPLANES - 1),
                            )
                        for i in range(NPLANES):
                            nc.tensor.matmul(
                                Mp[:], dM(g, i), K[i][:, sl],
                                start=(i == 0), stop=(i == NPLANES - 1),
                            )
                    nc.scalar.copy(y16[:, sl], Mp[:])
                    nc.vector.copy_predicated(
                        y16[:, sl],
                        g8[:, sl].bitcast(I16) if MASK == "act" else g8[:, sl],
                        Pp[:],
                    )

                if SPLIT_YDMA == "quarters":
                    q = W // 4
                    for qi in range(4):
                        nc.sync.dma_start(
                            yT[g0 * P : (g0 + 1) * P, qi * q : (qi + 1) * q],
                            y16[:, qi * q : (qi + 1) * q],
                        )
                elif SPLIT_YDMA:
                    h = W // 2
                    nc.sync.dma_start(yT[g0 * P : (g0 + 1) * P, :h], y16[:, :h])
                    nc.sync.dma_start(yT[g0 * P : (g0 + 1) * P, h:], y16[:, h:])
                else:
                    nc.sync.dma_start(
                        yT[g0 * P : (g0 + GPT) * P, :].rearrange(
                            "(gl p) b -> p gl b", p=P
                        ),
                        y16[:].rearrange("p (gl b) -> p gl b", gl=GPT),
                    )
    nc.compile()
    return nc


def _weights(coeff: np.ndarray):
    d = np.zeros((14, F), dtype=np.float64)
    c64 = coeff.astype(np.float64)
    for j in range(14):
        for r in range(5):
            n = j - r
            if 0 <= n < 10:
                d[j] += (-1) ** r * comb(4, r) / 6.0 * c64[:, n]
    s = 6.5 ** 3
    gA = np.zeros((NPLANES, F))
    gB = np.zeros((NPLANES, F))
    for i in range(NPLANES):
        gA[i] = -s * d[6 - i]
        gB[i] = -s * d[7 + i]
    d16 = np.zeros((G, P, 6 * P), dtype=np.float16)
    d32 = np.zeros((G, P, 8 * P), dtype=np.float32)
    rng = np.arange(P)
    for g in range(G):
        fsl = slice(g * P, (g + 1) * P)
        for i in FP16_PLANES:
            d16[g, rng, 2 * i * P + rng] = gB[i, fsl].astype(np.float16)
            d16[g, rng, (2 * i + 1) * P + rng] = gA[i, fsl].astype(np.float16)
        for i in FP32_PLANES:
            sgn = -1.0 if i in NEG32 else 1.0
            d32[g, rng, 2 * (i - 3) * P + rng] = (sgn * gB[i, fsl]).astype(np.float32)
            d32[g, rng, (2 * (i - 3) + 1) * P + rng] = (sgn * gA[i, fsl]).astype(np.float32)
    cst = np.zeros((P, 8), dtype=np.float32)
    cst[:, 0:4] = np.array(BT[3:], dtype=np.float32)
    cst[:, 4:7] = np.array(BT[:3], dtype=np.float32)
    return d16, d32, cst


def kernel(x: np.ndarray, coeff: np.ndarray) -> np.ndarray:
    x = np.ascontiguousarray(x, dtype=np.float32)
    coeff = np.ascontiguousarray(coeff, dtype=np.float32)
    assert x.shape == (B_FULL, F) and coeff.shape == (F, 10)

    if "nc" not in _CACHE:
        _CACHE["nc"] = _build_nc()
    nc = _CACHE["nc"]

    d16, d32, cst = _weights(coeff)

    in_maps = []
    for c in range(N_CORES):
        shard = np.ascontiguousarray(
            x[c * B_CORE : (c + 1) * B_CORE, :].T.astype(np.float16)
        )
        in_maps.append({"xT": shard, "d16": d16, "d32": d32, "cst": cst})

    trace = os.environ.get("BSPLINE_TRACE", "0") == "1"
    res = run_bass_kernel_spmd(
        nc, in_maps, core_ids=list(range(N_CORES)), trace=trace
    )
    _CACHE["last_result"] = res

    y = np.empty((B_FULL, F), dtype=np.float32)
    for c in range(N_CORES):
        y[c * B_CORE : (c + 1) * B_CORE, :] = res.results[c]["yT"].T.astype(np.float32)
    return y


# revision 44
# speedup vs baseline: 1.0256x; 1.0211x over previous
"""Trainium2 Bass kernel for nn_BSplineActivation (reflected truncated-power form).

Math: y[b,f] = sum_n B_n(x[b,f]) coeff[f,n], cubic B-splines on the uniform
grid linspace(-1,1,14).  In truncated-power form with u = 6.5(clip(x)+1):
  y = sum_{j=0..12} d_j (u-j)_+^3.
Adding the j=13 term (d_13 = coeff_9/6, zero on u<13) makes
  p(u) = sum_{j=0..13} d_j (u-j)^3 == 0  identically, so for u>6.5 the sum
collapses to minus-side powers of the *reflected* coordinate.  With
ax = |x| (clip unnecessary: planes vanish for ax>=1) and b~_m = m/6.5,
m in {0.5,...,6.5}:
  K_m = min((ax - b~_m)^3, 0)        -- 7 shared planes for BOTH branches
  x <  0:  y = sum_m gA_m K_m        (gA_m = -6.5^3 d_{6.5-m})
  x >= 0:  y = sum_m gB_m K_m        (gB_m = -6.5^3 d_{6.5+m})
Two PSUM chains (P = gB-weighted, M = gA-weighted) of per-feature diagonal
matmuls accumulate both branches; the finish selects P or M by sign(x).

Precision: planes m<=2.5 (small |gK| bound b~^3) run in fp16 end-to-end with
fp16 diagonal weights; planes m>=3.5 run fp32 with fp32r (tf32-rounded)
matmuls.  Measured rel-l2 vs the jax reference: 1.49e-2 (< 2e-2 gate).

Engine schedule (per [128,1024] group tile, cost-model balanced):
  ACT  abs, ungated squares for fp32 planes + plane 2, PSUM->fp16 drains
  DVE  fp16 plane ts/tt chain, fp16-N/fp32r-K for planes 3-4, sign mask,
       copy_predicated
  Pool N/K for planes 5-6
  PE   14 diag matmuls per 512-chunk (fp16 + fp32r, 1 cycle/row)
TimelineSim: 79005 ns/core (baseline kernel: 180808 ns, 2.29x).

Layout: features on partitions (8 groups of 128, processed 2 per [128,2048]
tile), batch on the free dim; pure data-parallel over batch across 8 cores.
Host sends per-core shards transposed and cast to fp16 (halves input DMA)
and pre-builds all diagonal weight tiles (zero engine cost, loaded once).
"""

import os
from math import comb

import numpy as np

import concourse.bacc as bacc
import concourse.bass as bass
import concourse.mybir as mybir
import concourse.tile as tile
from concourse.bass_utils import run_bass_kernel_spmd

N_CORES = 8
B_FULL, F = 8192, 1024
B_CORE = B_FULL // N_CORES  # 1024
P = 128
G = F // P  # 8
GPT = 1                      # groups per tile
NT = G // GPT                # 4 tiles per core
W = GPT * B_CORE             # 2048 free-dim cols per tile
CHUNK = 512
PSUM_BUFS = 2
CT_LATE = True
STT_POOL_CUBE = False  # stt on Pool is ISA-invalid on real codegen
FIRST_ABS_DVE = False
NPLANES = 7
BT = [(2 * i + 1) / 13.0 for i in range(NPLANES)]  # (i+0.5)/6.5
FP16_PLANES = (0, 1, 2)
FP32_PLANES = (3, 4, 5, 6)

FP32 = mybir.dt.float32
FP16 = mybir.dt.float16
F32R = mybir.dt.float32r
U8 = mybir.dt.uint8
I16 = mybir.dt.int16

Alu = mybir.AluOpType
Act = mybir.ActivationFunctionType

# fp32-plane engine routing: (N-producer, K-producer); S is always ACT.
# N "act" uses Relu(bt-ax) = -N; the host negates that plane's diag weights.
ROUTE32 = {
    3: ("dve16", "dve"),
    4: ("dve16", "dve"),
    5: ("pool", "pool"),
    6: ("pool", "pool"),
}
S16_ACT = (2,)
FIRST_SLIVER = False
LAST_TILE_DVE = False
CHAIN_ORDER = "interleaved"  # or "pm"
SPLIT_YDMA = True
MASK_EARLY = True
EMIT_ORDER = (5, 6, 0, 1, 2, 3, 4)
BUFS_X = 3
BUFS_W = 3
BUFS_Y = 3
BUFS_PL = 3
BUFS_TR = 2
BUFS_K32 = 2        # fp16 planes whose square runs on ACT (ungated from ax)
MASK = "dve"          # sign mask: "dve"|"pool" (is_ge) or "act" (relu, bitcast)
NEG32 = tuple(i for i in FP32_PLANES if ROUTE32[i][0] == "act")
_CACHE: dict = {}


def _build_nc() -> bass.Bass:
    nc = bacc.Bacc("TRN2", target_bir_lowering=False, debug=False)

    xT = nc.dram_tensor("xT", [F, B_CORE], FP16, kind="ExternalInput")
    # host-packed diagonal weights for ALL groups, partition-major:
    # d16: [P, G*6*P] fp16   (per group: P,M diag pairs for planes 0..2)
    # d32: [P, G*8*P] fp32r  (per group: P,M diag pairs for planes 3..6)
    d16 = nc.dram_tensor("d16", [G, P, 6 * P], FP16, kind="ExternalInput")
    d32 = nc.dram_tensor("d32", [G, P, 8 * P], F32R, kind="ExternalInput")
    cst = nc.dram_tensor("cst", [P, 8], FP32, kind="ExternalInput")
    yT = nc.dram_tensor("yT", [F, B_CORE], FP16, kind="ExternalOutput")

    with tile.TileContext(nc) as tc:
        with (
            tc.tile_pool(name="const", bufs=1) as const_pool,
            tc.tile_pool(name="xdata", bufs=BUFS_X) as x_pool,
            tc.tile_pool(name="wts", bufs=BUFS_W) as w_pool,
            tc.tile_pool(name="trans", bufs=BUFS_TR) as tr_pool,
            tc.tile_pool(name="k32p", bufs=BUFS_K32) as k32_pool,
            tc.tile_pool(name="plane", bufs=BUFS_PL) as pl_pool,
            tc.tile_pool(name="yout", bufs=BUFS_Y) as y_pool,
            tc.tile_pool(name="psum", bufs=PSUM_BUFS, space="PSUM") as psum_pool,
        ):
            ctall = const_pool.tile([P, 8], FP32, name="ctall")
            if not CT_LATE:
                nc.sync.dma_start(ctall[:], cst[:])



            for t in range(NT):
                g0 = t * GPT
                x16 = x_pool.tile([P, W], FP16, name="x16", tag="x16")
                if t == 0 and FIRST_SLIVER:
                    # sliver the first load so compute starts sooner
                    nc.sync.dma_start(x16[:, : W // 4], xT[g0 * P : (g0 + 1) * P, : W // 4])
                    nc.sync.dma_start(x16[:, W // 4 :], xT[g0 * P : (g0 + 1) * P, W // 4 :])
                else:
                    nc.sync.dma_start(
                        x16[:].rearrange("p (gl b) -> p gl b", gl=GPT),
                        xT[g0 * P : (g0 + GPT) * P, :].rearrange(
                            "(gl p) b -> p gl b", p=P
                        ),
                    )
                if CT_LATE and t == 0:
                    nc.sync.dma_start(ctall[:], cst[:])
                dg16 = w_pool.tile([P, 6 * P], FP16, name="dg16", tag="dg16")
                nc.sync.dma_start(dg16[:], d16[g0])
                dg32 = w_pool.tile([P, 8 * P], F32R, name="dg32", tag="dg32")
                nc.sync.dma_start(dg32[:], d32[g0])

                def dP(g, i):
                    if i in FP16_PLANES:
                        return dg16[:, (2 * i) * P : (2 * i + 1) * P]
                    c = 2 * (i - 3) * P
                    return dg32[:, c : c + P]

                def dM(g, i):
                    if i in FP16_PLANES:
                        return dg16[:, (2 * i + 1) * P : (2 * i + 2) * P]
                    c = (2 * (i - 3) + 1) * P
                    return dg32[:, c : c + P]

                ax = pl_pool.tile([P, W], FP16, name="ax", tag="ax")
                if t == 0 and FIRST_ABS_DVE:
                    negx = tr_pool.tile([P, W], FP16, name="negx", tag="negx")
                    nc.vector.tensor_scalar(negx[:], x16[:], -1.0, None, Alu.mult)
                    nc.vector.tensor_max(ax[:], x16[:], negx[:])
                elif t == 0 and FIRST_SLIVER:
                    nc.scalar.activation(ax[:, : W // 4], x16[:, : W // 4], Act.Abs)
                    nc.scalar.activation(ax[:, W // 4 :], x16[:, W // 4 :], Act.Abs)
                else:
                    nc.scalar.activation(ax[:], x16[:], Act.Abs)

                K = {}

                def emit16(i):
                    n = tr_pool.tile([P, W], FP16, name=f"n{i}", tag=f"n{i}")
                    nc.vector.tensor_scalar(
                        n[:], ax[:], BT[i], 0.0, Alu.subtract, Alu.min
                    )
                    s = tr_pool.tile([P, W], FP16, name=f"s{i}", tag=f"s{i}")
                    if i in S16_ACT:
                        nc.scalar.activation(
                            s[:], ax[:], Act.Square,
                            bias=ctall[:, 4 + i : 5 + i], scale=-1.0,
                        )
                    else:
                        nc.vector.tensor_tensor(s[:], n[:], n[:], Alu.mult)
                    k = pl_pool.tile([P, W], FP16, name=f"k{i}", tag=f"k{i}")
                    nc.vector.tensor_tensor(k[:], s[:], n[:], Alu.mult)
                    K[i] = k
                if MASK_EARLY:
                    g8 = pl_pool.tile([P, W], I16, name="g8", tag="g8")
                    geng = nc.vector if MASK == "dve" else nc.gpsimd
                    geng.tensor_scalar(g8[:], x16[:], 0.0, 1.0, Alu.is_ge, Alu.mult)
                def emit32(i):
                    rn, rk = ROUTE32[i]
                    ndt = FP16 if rn == "dve16" else FP32
                    n = tr_pool.tile([P, W], ndt, name=f"n{i}", tag=f"n{i}")
                    if rn == "act":
                        # n = relu(bt - ax) = -N ; diag weights negated on host
                        nc.scalar.activation(
                            n[:], ax[:], Act.Relu,
                            bias=ctall[:, i - 3 : i - 2], scale=-1.0,
                        )
                    elif rn == "splitdp":
                        h = W // 2
                        nc.vector.tensor_scalar(
                            n[:, :h], ax[:, :h], BT[i], 0.0, Alu.subtract, Alu.min
                        )
                        nc.gpsimd.tensor_scalar(
                            n[:, h:], ax[:, h:], BT[i], 0.0, Alu.subtract, Alu.min
                        )
                    else:
                        eng = nc.gpsimd if rn == "pool" else nc.vector
                        eng.tensor_scalar(n[:], ax[:], BT[i], 0.0, Alu.subtract, Alu.min)
                    s = tr_pool.tile([P, W], FP32, name=f"s{i}", tag=f"s{i}")
                    nc.scalar.activation(
                        s[:], ax[:], Act.Square, bias=ctall[:, i - 3 : i - 2], scale=-1.0
                    )
                    k = k32_pool.tile([P, W], F32R, name=f"k{i}", tag=f"k{i}")
                    if rk == "pool" and STT_POOL_CUBE:
                        # stt lowers to TensorScalarPtr: Pool efficiency 0.6
                        # instead of TensorTensor-Multiply's 0.42
                        nc.gpsimd.scalar_tensor_tensor(
                            k[:], s[:], 1.0, n[:], Alu.mult, Alu.mult
                        )
                    else:
                        eng = nc.vector if rk == "dve" else nc.gpsimd
                        eng.tensor_tensor(k[:], s[:], n[:], Alu.mult)
                    K[i] = k

                for i in EMIT_ORDER:
                    (emit16 if i in FP16_PLANES else emit32)(i)

                if not MASK_EARLY:
                    if MASK == "act":
                        g8 = pl_pool.tile([P, W], FP16, name="g8", tag="g8")
                        nc.scalar.activation(g8[:], x16[:], Act.Relu)
                    else:
                        g8 = pl_pool.tile([P, W], I16, name="g8", tag="g8")
                        geng = nc.vector if MASK == "dve" else nc.gpsimd
                        geng.tensor_scalar(g8[:], x16[:], 0.0, 1.0, Alu.is_ge, Alu.mult)
                y16 = y_pool.tile([P, W], FP16, name="y16", tag="y16")

                for ch in range(W // CHUNK):
                    gl = ch // (B_CORE // CHUNK)     # which group in the tile
                    g = g0 + gl
                    Pp = psum_pool.tile([P, CHUNK], FP32, name=f"Pp{ch}", tag=f"Pp{ch}")
                    Mp = psum_pool.tile([P, CHUNK], FP32, name=f"Mp{ch}", tag=f"Mp{ch}")
                    sl = slice(ch * CHUNK, (ch + 1) * CHUNK)
                    if CHAIN_ORDER == "interleaved":
                        for i in range(NPLANES):
                            nc.tensor.matmul(
                                Pp[:], dP(g, i), K[i][:, sl],
                                start=(i == 0), stop=(i == NPLANES - 1),
                            )
                            nc.tensor.matmul(
                                Mp[:], dM(g, i), K[i][:, sl],
                                start=(i == 0), stop=(i == NPLANES - 1),
                            )
                    else:
                        for i in range(NPLANES):
                            nc.tensor.matmul(
                                Pp[:], dP(g, i), K[i][:, sl],
                                start=(i == 0), stop=(i == NPLANES - 1),
                            )
                        for i in range(NPLANES):
                            nc.tensor.matmul(
                                Mp[:], dM(g, i), K[i][:, sl],
                                start=(i == 0), stop=(i == NPLANES - 1),
                            )
                    nc.scalar.copy(y16[:, sl], Mp[:])
                    nc.vector.copy_predicated(
                        y16[:, sl],
                        g8[:, sl].bitcast(I16) if MASK == "act" else g8[:, sl],
                        Pp[:],
                    )

                if SPLIT_YDMA == "quarters":
                    q = W // 4
                    for qi in range(4):
                        nc.sync.dma_start(
                            yT[g0 * P : (g0 + 1) * P, qi * q : (qi + 1) * q],
                            y16[:, qi * q : (qi + 1) * q],
                        )
                elif SPLIT_YDMA:
                    h = W // 2
                    nc.sync.dma_start(yT[g0 * P : (g0 + 1) * P, :h], y16[:, :h])
                    nc.sync.dma_start(yT[g0 * P : (g0 + 1) * P, h:], y16[:, h:])
                else:
                    nc.sync.dma_start(
                        yT[g0 * P : (g0 + GPT) * P, :].rearrange(
                            "(gl p) b -> p gl b", p=P
                        ),
                        y16[:].rearrange("p (gl b) -> p gl b", gl=GPT),
                    )
    nc.compile()
    return nc


def _weights(coeff: np.ndarray):
    d = np.zeros((14, F), dtype=np.float64)
    c64 = coeff.astype(np.float64)
    for j in range(14):
        for r in range(5):
            n = j - r
            if 0 <= n < 10:
                d[j] += (-1) ** r * comb(4, r) / 6.0 * c64[:, n]
    s = 6.5 ** 3
    gA = np.zeros((NPLANES, F))
    gB = np.zeros((NPLANES, F))
    for i in range(NPLANES):
        gA[i] = -s * d[6 - i]
        gB[i] = -s * d[7 + i]
    d16 = np.zeros((G, P, 6 * P), dtype=np.float16)
    d32 = np.zeros((G, P, 8 * P), dtype=np.float32)
    rng = np.arange(P)
    for g in range(G):
        fsl = slice(g * P, (g + 1) * P)
        for i in FP16_PLANES:
            d16[g, rng, 2 * i * P + rng] = gB[i, fsl].astype(np.float16)
            d16[g, rng, (2 * i + 1) * P + rng] = gA[i, fsl].astype(np.float16)
        for i in FP32_PLANES:
            sgn = -1.0 if i in NEG32 else 1.0
            d32[g, rng, 2 * (i - 3) * P + rng] = (sgn * gB[i, fsl]).astype(np.float32)
            d32[g, rng, (2 * (i - 3) + 1) * P + rng] = (sgn * gA[i, fsl]).astype(np.float32)
    cst = np.zeros((P, 8), dtype=np.float32)
    cst[:, 0:4] = np.array(BT[3:], dtype=np.float32)
    cst[:, 4:7] = np.array(BT[:3], dtype=np.float32)
    return d16, d32, cst


def kernel(x: np.ndarray, coeff: np.ndarray) -> np.ndarray:
    x = np.ascontiguousarray(x, dtype=np.float32)
    coeff = np.ascontiguousarray(coeff, dtype=np.float32)
    assert x.shape == (B_FULL, F) and coeff.shape == (F, 10)

    if "nc" not in _CACHE:
        _CACHE["nc"] = _build_nc()
    nc = _CACHE["nc"]

    d16, d32, cst = _weights(coeff)

    in_maps = []
    for c in range(N_CORES):
        shard = np.ascontiguousarray(
            x[c * B_CORE : (c + 1) * B_CORE, :].T.astype(np.float16)
        )
        in_maps.append({"xT": shard, "d16": d16, "d32": d32, "cst": cst})

    trace = os.environ.get("BSPLINE_TRACE", "0") == "1"
    res = run_bass_kernel_spmd(
        nc, in_maps, core_ids=list(range(N_CORES)), trace=trace
    )
    _CACHE["last_result"] = res

    y = np.empty((B_FULL, F), dtype=np.float32)
    for c in range(N_CORES):
        y[c * B_CORE : (c + 1) * B_CORE, :] = res.results[c]["yT"].T.astype(np.float32)
    return y
